# revision 4
# baseline (speedup 1.0000x reference)
"""Trainium2 Bass kernel for nn_ChannelAttention (sparse_attention, memory regime).

Reference computation (per batch b, with C=64 channels, N=H*W=65536 positions):
    v        = x.reshape(B, C, N)
    inv[n]   = 1 / (||v[:, n]||_2 + 1e-6)
    qn       = v * inv                       (l2-normalized over channels)
    q_sum[n] = sum_c qn[c, n] = inv[n] * value_sum[n]
    tailor[c]= 1 / (N + sum_n qn[c,n]*q_sum[n] + 1e-6)
    matrix   = v @ qn^T                       [C, C]  (= sum_n v_c v_m inv, symmetric)
    msum     = value_sum[None, :] + matrix @ qn
    out      = x + gamma * (msum * tailor[:, None])

Sharding: data-parallel over batch. 16 batches / 8 cores = 2 batches per core,
stacked on the partition axis (64 channels each -> 128 partitions). No collectives.

Per-core algorithm (all on one NeuronCore, x_core = [128, 65536] fp32):
  Phase A (chunked/position-major read): x is DMA'd in "chunked" layout
    tile[p, c, j] = x[c, s*L + p*J + j] so positions land on partitions with
    1KB-contiguous DMA segments and NO on-chip transposes. Stats (value_sum,
    sum-of-squares over channels) are bf16 TT-trees over the free c axis; the
    weighted Gram matrix + tailor column are built by 128-position "j-matmuls"
    on the PE accumulating into one PSUM tile:
        gram[c, m] += sum_p X'[p, c, j] * X'[p, m, j],  X' = x * sqrt(inv)
        gram[c, 64] += sum_p X'[p, c, j] * u'[p, j],    u' = inv^1.5 * vs
  Interlude: tailor/A = gamma*tailor, the block-diagonal stationary
    M2[m, c] = matrix[m, c] * A[c], the rank-2 stationaries.
  Phase B (natural read): out = x + (M2 @ x + A (x) vs') * bcast(inv) computed
    as: main fp32 matmul + K=2 fp32r matmul (adds A[c]*vs'[n]) into one PSUM,
    a K=2 fp32r "broadcast matmul" replicating inv across partitions in a
    second PSUM, ACT evacuation, then two elementwise ops (DVE/GPSIMD).
"""

import sys
import os

for _p in ("/opt/trn_rl_repo", "/root/.axon_site/_ro/trn_rl_repo"):
    if os.path.isdir(_p) and _p not in sys.path:
        sys.path.insert(0, _p)

import numpy as np
from contextlib import ExitStack

import concourse.bass as bass
import concourse.tile as tile
import concourse.mybir as mybir
from concourse.vector_clock import ScopedClock
from concourse.bass_utils import run_bass_kernel_spmd

AF = mybir.ActivationFunctionType
F32 = mybir.dt.float32
F32R = mybir.dt.float32r
BF16 = mybir.dt.bfloat16

# Problem geometry (hardcoded for nn_ChannelAttention_64493228916840)
B_TOTAL = 16
C = 64            # channels per batch
H = W = 256
N = H * W         # 65536 positions
NCORES = 8
BPC = B_TOTAL // NCORES   # 2 batches per core
P = 128                   # partitions (= BPC * C)
L = 16384                 # positions per slab
S = N // L                # 4 slabs per batch
J = L // P                # 128 positions-per-partition per slab
CHUNK = 512               # phase-B matmul free size
PAIR = 2 * CHUNK          # coalesced load/store size
NPAIR = N // PAIR         # 64
EPS = 1e-6

MAX_WAITS = 1


class PatchedTileContext(tile.TileContext):
    """Walrus CoreV3 in this container accepts at most one semaphore wait per
    instruction; hoist excess waits onto NoOp carriers on the same engine."""

    def _add_instruction(self, inst):
        si = getattr(inst, "sync_info", None)
        if si is not None and si.on_wait and len(si.on_wait) > MAX_WAITS:
            waits = list(si.on_wait)
            si.on_wait = waits[-MAX_WAITS:]
            for w in waits[:-MAX_WAITS]:
                nop = mybir.InstNoOp(
                    name=self.nc.get_next_instruction_name(), ins=[], outs=[]
                )
                nop.engine = inst.engine
                nop.sync_info = mybir.SyncInfo(on_wait=[w], on_update=[])
                super()._add_instruction(nop)
        super()._add_instruction(inst)

    def _drain_and_barrier(self, tick_clock, wait_clock):
        nc = self.nc
        drain_inst = nc.sync.drain()
        wait_clock.add_sem_waits(
            drain_inst.ins, ScopedClock({None: tick_clock.global_clock})
        )
        inst = drain_inst.ins
        si = inst.sync_info
        if si is not None and si.on_wait and len(si.on_wait) > MAX_WAITS:
            waits = list(si.on_wait)
            si.on_wait = waits[:MAX_WAITS]
            for w in waits[MAX_WAITS:]:
                nop = nc.sync.nop(nofuse=True, hint="drain_waitsplit")
                nsi = nop.ins.sync_info
                if nsi is None:
                    nop.ins.sync_info = mybir.SyncInfo(on_wait=[w], on_update=[])
                else:
                    nsi.on_wait = [w]
        nc.all_engine_barrier()
        assert self.sems is not None
        popped = nc._tile_sem_poison_stack.pop()
        assert popped is self._sem_poison
        nc.clear_and_free_semaphores(list(self.sems.allocated().values()))
        nc.all_engine_barrier()


def _bcast_partitions(ap, num):
    """DMA source AP replicating partition 0 across `num` partitions."""
    return bass.AP(tensor=ap.tensor, offset=ap.offset, ap=[[0, num]] + list(ap.ap)[1:])


def build_program():
    nc = bass.Bass("TRN2", target_bir_lowering=False, debug=False)
    x_d = nc.dram_tensor("x", [P, N], F32, kind="ExternalInput").ap()
    g_d = nc.dram_tensor("gamma", [1, 1], F32, kind="ExternalInput").ap()
    y_d = nc.dram_tensor("y", [P, N], F32, kind="ExternalOutput").ap()

    with PatchedTileContext(nc) as tc:
        with ExitStack() as octx:
            persist = octx.enter_context(tc.tile_pool(name="persist", bufs=1))
            gram_pool = octx.enter_context(
                tc.tile_pool(name="gram_ps", bufs=1, space="PSUM")
            )
            gram_ps = gram_pool.tile([P, C + 1], F32)

            # persistent per-(b, s) stats in chunked layout, fp32r for phase-B matmuls
            vsr = {}
            invr = {}
            for b in range(BPC):
                for s in range(S):
                    vsr[(b, s)] = persist.tile([P, J], F32R, name=f"vsr_{b}_{s}", tag=f"vsr_{b}_{s}")
                    invr[(b, s)] = persist.tile([P, J], F32R, name=f"invr_{b}_{s}", tag=f"invr_{b}_{s}")

            # ---------------- PHASE A ----------------
            with ExitStack() as actx:
                apool = actx.enter_context(tc.tile_pool(name="phaseA", bufs=2))
                asc = actx.enter_context(tc.tile_pool(name="phaseA_scratch", bufs=2))

                for s in range(S):
                    for b in range(BPC):
                        # chunked bf16 cast-load: xcb[p, c, j] = x[64b+c, s*L + p*J + j]
                        xcb = apool.tile([P, C, J], BF16, tag="xcb")
                        src = x_d[C * b : C * (b + 1), L * s : L * (s + 1)].rearrange(
                            "c (p j) -> p c j", p=P
                        )
                        nc.gpsimd.dma_start(out=xcb, in_=src)

                        # squares (ACT) for the sum-of-squares tree
                        sq = asc.tile([P, C, J], BF16, tag="sq")
                        nc.scalar.activation(out=sq, in_=xcb, func=AF.Square)

                        # bf16 TT-add trees over the channel axis
                        def tree(src_t, out_t, eng, tagp):
                            cur = src_t
                            width = C
                            while width > 2:
                                width //= 2
                                nxt = asc.tile([P, width, J], BF16, tag=f"{tagp}{width}")
                                eng.tensor_add(
                                    out=nxt,
                                    in0=cur[:, 0:width, :],
                                    in1=cur[:, width : 2 * width, :],
                                )
                                cur = nxt
                            eng.tensor_add(
                                out=out_t, in0=cur[:, 0, :], in1=cur[:, 1, :]
                            )

                        vs = asc.tile([P, J], F32, tag="vs")
                        tree(xcb, vs, nc.vector, "tv")
                        ss = asc.tile([P, J], F32, tag="ss")
                        tree(sq, ss, nc.gpsimd, "ts")

                        # finishers: norme = sqrt(ss)+eps; inv = 1/norme; vs' = vs*norme
                        norme = asc.tile([P, J], F32, tag="norme")
                        nc.scalar.activation(out=norme, in_=ss, func=AF.Sqrt)
                        nc.vector.tensor_scalar_add(out=norme, in0=norme, scalar1=float(EPS))
                        inv = asc.tile([P, J], F32, tag="inv")
                        nc.vector.reciprocal(out=inv, in_=norme)
                        nc.vector.tensor_mul(out=vsr[(b, s)], in0=vs, in1=norme)
                        nc.vector.tensor_copy(out=invr[(b, s)], in_=inv)
                        sinv = asc.tile([P, J], F32, tag="sinv")
                        nc.scalar.activation(out=sinv, in_=inv, func=AF.Sqrt)
                        sinvb = asc.tile([P, J], BF16, tag="sinvb")
                        nc.vector.tensor_copy(out=sinvb, in_=sinv)

                        # X'aug: cols 0..63 = x*sqrt(inv) (bf16), col 64 = u' = inv^1.5*vs
                        xaug = apool.tile([P, C + 1, J], BF16, tag="xaug")
                        sinv_b = bass.AP(
                            tensor=sinvb.tensor,
                            offset=sinvb.offset,
                            ap=[list(sinvb.ap)[0], [0, C]] + list(sinvb.ap)[1:],
                        )
                        nc.vector.tensor_mul(out=xaug[:, 0:C, :], in0=xcb, in1=sinv_b)
                        i15 = asc.tile([P, J], F32, tag="i15")
                        nc.vector.tensor_mul(out=i15, in0=inv, in1=sinv)
                        nc.vector.tensor_mul(out=xaug[:, C, :], in0=i15, in1=vs)

                        # gram j-matmuls accumulate over all slabs
                        for j in range(J):
                            nc.tensor.matmul(
                                gram_ps[C * b : C * (b + 1), :],
                                lhsT=xaug[:, 0:C, j],
                                rhs=xaug[:, :, j],
                                start=(s == 0 and j == 0),
                                stop=(s == S - 1 and j == J - 1),
                            )

            # ---------------- INTERLUDE ----------------
            inter = octx.enter_context(tc.tile_pool(name="inter", bufs=1))
            gram_sb = inter.tile([P, C + 1], F32)
            nc.vector.tensor_copy(out=gram_sb, in_=gram_ps)

            # tailor = 1/(N + t + eps), A = gamma * tailor   (per-partition [128,1])
            tail = inter.tile([P, 1], F32)
            nc.vector.tensor_scalar_add(
                out=tail, in0=gram_sb[:, C : C + 1], scalar1=float(N + EPS)
            )
            nc.vector.reciprocal(out=tail, in_=tail)
            gam = inter.tile([P, 1], F32)
            nc.sync.dma_start(out=gam, in_=_bcast_partitions(g_d, P))
            A_t = inter.tile([P, 1], F32)
            nc.vector.tensor_mul(out=A_t, in0=tail, in1=gam)

            # A as a free-dim row (bounced through DRAM for partition-broadcast)
            arow = inter.tile([1, P], F32)
            nc.sync.dma_start(
                out=arow.rearrange("c (p j) -> c p j", p=P), in_=A_t
            )
            arow_d = nc.dram_tensor("arow_scratch", [1, P], F32).ap()
            nc.sync.dma_start(out=arow_d, in_=arow)
            abc = inter.tile([P, P], F32)
            nc.sync.dma_start(out=abc, in_=_bcast_partitions(arow_d, P))

            # M2[m, c] = matrix[m, c] * A[c] block-diagonally (matrix symmetric)
            m2 = inter.tile([P, P], F32)
            nc.vector.memset(m2, 0.0)
            nc.vector.tensor_mul(
                out=m2[0:C, 0:C], in0=gram_sb[0:C, 0:C], in1=abc[0:C, 0:C]
            )
            nc.vector.tensor_mul(
                out=m2[C:P, C:P], in0=gram_sb[C:P, 0:C], in1=abc[C:P, C:P]
            )

            # rank-2 stationaries: lhsT2[s, c] = A[c] masked by batch, ones2 likewise
            onesrow = inter.tile([1, P], F32)
            nc.vector.memset(onesrow, 1.0)
            l2f = inter.tile([2, P], F32)
            nc.vector.memset(l2f, 0.0)
            nc.sync.dma_start(out=l2f[0:1, 0:C], in_=arow[0:1, 0:C])
            nc.sync.dma_start(out=l2f[1:2, C:P], in_=arow[0:1, C:P])
            lhsT2 = inter.tile([2, P], F32R)
            nc.vector.tensor_copy(out=lhsT2, in_=l2f)
            o2f = inter.tile([2, P], F32)
            nc.vector.memset(o2f, 0.0)
            nc.sync.dma_start(out=o2f[0:1, 0:C], in_=onesrow[0:1, 0:C])
            nc.sync.dma_start(out=o2f[1:2, C:P], in_=onesrow[0:1, C:P])
            ones2 = inter.tile([2, P], F32R)
            nc.vector.tensor_copy(out=ones2, in_=o2f)

            # ---------------- PHASE B ----------------
            bpool = octx.enter_context(tc.tile_pool(name="phaseB", bufs=3))
            stpool = octx.enter_context(tc.tile_pool(name="stageB", bufs=3))
            psB = octx.enter_context(tc.tile_pool(name="psB", bufs=3, space="PSUM"))
            psBC = octx.enter_context(tc.tile_pool(name="psBC", bufs=3, space="PSUM"))

            for u in range(NPAIR):
                n0 = u * PAIR
                s = n0 // L
                local = n0 - s * L
                p0 = local // J          # 8 j-rows per pair per batch

                xn = bpool.tile([P, PAIR], F32, tag="xn")
                nc.sync.dma_start(out=xn, in_=x_d[:, n0 : n0 + PAIR])

                stage_vs = stpool.tile([2, PAIR], F32R, tag="stage_vs")
                stage_inv = stpool.tile([2, PAIR], F32R, tag="stage_inv")
                for b in range(BPC):
                    dst_v = stage_vs[b : b + 1, :].rearrange("c (p j) -> c p j", p=8)
                    nc.scalar.dma_start(out=dst_v, in_=vsr[(b, s)][p0 : p0 + 8, :])
                    dst_i = stage_inv[b : b + 1, :].rearrange("c (p j) -> c p j", p=8)
                    nc.scalar.dma_start(out=dst_i, in_=invr[(b, s)][p0 : p0 + 8, :])

                out_sb = bpool.tile([P, PAIR], F32, tag="out_sb")
                for h in range(2):
                    lo = h * CHUNK
                    hi = lo + CHUNK
                    ps_main = psB.tile([P, CHUNK], F32, tag="ps_main")
                    nc.tensor.matmul(
                        ps_main, lhsT=m2, rhs=xn[:, lo:hi], start=True, stop=False
                    )
                    nc.tensor.matmul(
                        ps_main,
                        lhsT=lhsT2,
                        rhs=stage_vs[:, lo:hi],
                        start=False,
                        stop=True,
                    )
                    ps_bc = psBC.tile([P, CHUNK], F32, tag="ps_bc")
                    nc.tensor.matmul(
                        ps_bc, lhsT=ones2, rhs=stage_inv[:, lo:hi], start=True, stop=True
                    )
                    bci = bpool.tile([P, CHUNK], F32, tag="bci")
                    nc.scalar.copy(out=bci, in_=ps_bc)
                    tmp = bpool.tile([P, CHUNK], F32, tag="tmp")
                    nc.vector.tensor_mul(out=tmp, in0=ps_main, in1=bci)
                    k = 2 * u + h
                    if k % 8 < 3:
                        nc.vector.tensor_add(
                            out=out_sb[:, lo:hi], in0=tmp, in1=xn[:, lo:hi]
                        )
                    else:
                        nc.gpsimd.tensor_add(
                            out=out_sb[:, lo:hi], in0=tmp, in1=xn[:, lo:hi]
                        )
                nc.scalar.dma_start(out=y_d[:, n0 : n0 + PAIR], in_=out_sb)

    return nc


_NC_CACHE = None


def kernel(x: np.ndarray, gamma: np.ndarray) -> np.ndarray:
    global _NC_CACHE
    assert x.shape == (B_TOTAL, C, H, W), x.shape
    x = np.ascontiguousarray(x, dtype=np.float32)
    gamma = np.asarray(gamma, dtype=np.float32).reshape(1, 1)

    if _NC_CACHE is None:
        _NC_CACHE = build_program()
    nc = _NC_CACHE

    xf = x.reshape(B_TOTAL, C, N)
    in_maps = []
    for i in range(NCORES):
        shard = np.ascontiguousarray(
            xf[BPC * i : BPC * (i + 1)].reshape(P, N)
        )
        in_maps.append({"x": shard, "gamma": gamma})

    res = run_bass_kernel_spmd(nc, in_maps, list(range(NCORES)))
    out = np.empty((B_TOTAL, C, N), dtype=np.float32)
    for i in range(NCORES):
        out[BPC * i : BPC * (i + 1)] = res.results[i]["y"].reshape(BPC, C, N)
    return out.reshape(B_TOTAL, C, H, W)
